# revision 2
# baseline (speedup 1.0000x reference)
"""Trainium2 Bass kernel for nn_AlignModule (QAConv correlation + PAM), fp8.

Reference computation (B=32, C=512, H=24, W=8, hw=192, C8=64):
  xf = x.reshape(B, C, hw)
  score[g,p,n,m] = sum_c xf[g,c,m] * xf[p,c,n]          # [B,B,hw,hw]
  kernel_max[g,p,n] = max_m score[g,p,n,m]              # [B,B,hw]
  q = Wq @ xf[b] + bq; k = Wk @ xf[b] + bk              # [B,C8,hw]
  energy[b,m,n] = sum_q q[b,q,m] k[b,q,n]
  pos_max[b,m] = max_n energy[b,m,n]                    # [B,hw]
  out = concat([kernel_max, pos_max[None]], axis=0)     # [B+1,B,hw]

Sharding: data-parallel over g across 8 cores (4 images each).  Each core
receives the full x as [C, B*hw] fp8e4m3, rolled so its own images occupy
columns [0, 768).

QAConv matmuls run in fp8 DoubleRow perf mode: each instruction contracts
256 channels (128 partitions x 2 double-rows) at 0.5 cycles per moving
column -- 4x the fp32r rate of the previous kernel.  The 2e-2 relative
error budget (vs output absmax ~639) covers fp8 quantization (~1.2%).

The per-block max reductions (96 psum blocks of [128, 2, 192] per core)
are split across three engines so they keep pace with the PE:
  - DVE: tensor_tensor_reduce directly from PSUM (per-g halves)
  - Pool: tensor_max (halves) PSUM -> SBUF bf16, finished by a batched
    DVE bf16 max tree (2x_1p mode) 96 -> 1
  - Act: copy PSUM -> SBUF bf16, same DVE tree from 192
Stage-2 results land in a path-permuted column order; the host unpermutes.
"""

import numpy as np
import ml_dtypes

import concourse.bass as bass
import concourse.mybir as mybir
import concourse.tile as tile
from concourse import bacc
from concourse.bass_utils import run_bass_kernel_spmd
from concourse.masks import make_identity

B = 32
C = 512
HW = 192
C8 = 64
N_CORES = 8
GPC = B // N_CORES            # images per core (4)
FLAT = B * HW                 # flattened (p, n) axis (6144)
GROLL = GPC * HW              # per-core roll step (768)
NCH = FLAT // GROLL           # column chunks of 768 (8)
NJ = FLAT // 128              # stationary 128-column blocks (48)
JPC = GROLL // 128            # j blocks per column chunk (6)
WSCALE = 64.0                 # host premultiplier for Wq/Wk before fp8 cast

F32 = mybir.dt.float32
BF16 = mybir.dt.bfloat16
F8 = mybir.dt.float8e4
AX = mybir.AxisListType.X
MAX = mybir.AluOpType.max
DR = mybir.MatmulPerfMode.DoubleRow
IDENT = mybir.ActivationFunctionType.Identity
COPY = mybir.ActivationFunctionType.Copy
HAS_BIAS = [False]  # set by kernel() before (re)build

NP_F8 = ml_dtypes.float8_e4m3
NP_BF16 = ml_dtypes.bfloat16

# reduce-path assignment over the 48 j blocks:
# D = DVE tensor_tensor_reduce direct, P = Pool tensor_max + DVE tree,
# A = Act copy + DVE tree.  D js early (fill DVE while trees wait for
# egress); A js kept away from the tail (their trees are heavy).
import os
_D_POS_BY_N = {
    12: [0, 3, 8, 11, 16, 19, 24, 27, 32, 37, 42, 47],
    14: [0, 3, 6, 11, 14, 17, 22, 25, 28, 33, 36, 39, 44, 47],
    16: [0, 3, 6, 9, 12, 15, 18, 21, 24, 27, 30, 35, 38, 41, 44, 47],
}
D_POS = _D_POS_BY_N[int(os.environ.get("K_ND", "14"))]


def _default_sched():
    # Walrus-legal reduce engines are only DVE and Act: D = DVE direct
    # tensor_reduce, A = Act copy (full-bank pairs) + batched DVE bf16
    # tree.  A js come in consecutive even-length runs (pairs share 3
    # psum banks); D placement keeps runs even and covers the tail.
    out = ["A"] * NJ
    for p in D_POS:
        out[p] = "D"
    return "".join(out)


_SCHED = os.environ.get("K_SCHED") or _default_sched()
PATHS = list(_SCHED)
assert len(PATHS) == NJ, (len(PATHS), _SCHED)
N_D = PATHS.count("D")
N_P = PATHS.count("P")
N_A = PATHS.count("A")
# stage-2 flush thresholds (cumulative slots; flush when slot count hits one)
def _flushes(n, k):
    nb = max(1, round(n / k))
    out = [round(n * (i + 1) / nb) for i in range(nb)]
    return out
N_PAIR = N_A // 2
A_FLUSH = [3, 6, 9, 12, 14, 15, 16, 17][:N_PAIR]
if A_FLUSH[-1] != N_PAIR:
    A_FLUSH = _flushes(N_PAIR, 3)

_COMPILED = None
# res column order: D js first, then P js, then A js (each in j order)
ORDER = ([j for j, p in enumerate(PATHS) if p == "D"]
         + [j for j, p in enumerate(PATHS) if p == "P"]
         + [j for j, p in enumerate(PATHS) if p == "A"])
COL_OF_J = {j: c for c, j in enumerate(ORDER)}


def _tree_max(nc, c, width):
    """In-place DVE bf16 max tree over the last axis: width -> 3."""
    w = width
    while w > 3:
        h = w // 2
        nc.vector.tensor_max(c[:, :, :, 0:h], c[:, :, :, 0:h], c[:, :, :, h:w])
        w = h


def _build():
    nc = bacc.Bacc("TRN2", target_bir_lowering=False, debug=False)

    x8 = nc.dram_tensor("x8", [C, FLAT], F8, kind="ExternalInput").ap()
    wq8 = nc.dram_tensor("wq8", [C, C8], F8, kind="ExternalInput").ap()
    wk8 = nc.dram_tensor("wk8", [C, C8], F8, kind="ExternalInput").ap()
    bqw = nc.dram_tensor("bqw", [2, C8], F8, kind="ExternalInput").ap()
    bkw = nc.dram_tensor("bkw", [2, C8], F8, kind="ExternalInput").ap()
    # kmax_bf[c, g, t]: kernel_max[g, ORDER[c]*128 + t] (rolled flat order)
    kmax_bf = nc.dram_tensor("kmax_bf", [NJ, GPC, 128], BF16,
                             kind="ExternalOutput").ap()
    # pmax_bf[2*b+h, t]: pos_max[b, h*128 + t] (h=1 valid for t < 64)
    pmax_bf = nc.dram_tensor("pmax_bf", [2 * GPC, 128], BF16,
                             kind="ExternalOutput").ap()

    with tile.TileContext(nc) as tc:
        with (
            tc.tile_pool(name="xpool", bufs=1) as xpool,
            tc.tile_pool(name="wpool", bufs=1) as wpool,
            tc.tile_pool(name="stage", bufs=1) as stage,
            tc.tile_pool(name="qad_psum", bufs=2, space="PSUM") as qad_psum,
            tc.tile_pool(name="qaa_psum", bufs=4, space="PSUM") as qaa_psum,
            tc.tile_pool(name="pam_psum", bufs=2, space="PSUM") as pam_psum,
        ):
            # ---- x tiles [128, 2, 768] fp8 per (cc, ch); channel
            # c = cc*256 + i*128 + k lives at tile[k, i, :] ----
            xt = [[None] * NCH for _ in range(2)]
            wq_sb, wk_sb = [None, None], [None, None]

            def load_x(cc, ch):
                t = xpool.tile([128, 2, GROLL], F8, tag=f"x_{cc}_{ch}", name=f"x_{cc}_{ch}")
                nc.sync.dma_start(
                    t[:],
                    x8[cc * 256:(cc + 1) * 256,
                       ch * GROLL:(ch + 1) * GROLL].rearrange(
                        "(i p) c -> p i c", p=128),
                )
                xt[cc][ch] = t

            load_x(0, 0)
            load_x(1, 0)
            for cc2 in range(2):
                wq_sb[cc2] = wpool.tile([128, 2, C8], F8, tag=f"wq_{cc2}", name=f"wq_{cc2}")
                nc.sync.dma_start(
                    wq_sb[cc2][:],
                    wq8[cc2 * 256:(cc2 + 1) * 256, :].rearrange(
                        "(i p) q -> p i q", p=128))
                wk_sb[cc2] = wpool.tile([128, 2, C8], F8, tag=f"wk_{cc2}", name=f"wk_{cc2}")
                nc.sync.dma_start(
                    wk_sb[cc2][:],
                    wk8[cc2 * 256:(cc2 + 1) * 256, :].rearrange(
                        "(i p) q -> p i q", p=128))
            bqw_sb = wpool.tile([1, 2, C8], F8)
            nc.sync.dma_start(
                bqw_sb[:], bqw.rearrange("(p i) q -> p i q", p=1))
            bkw_sb = wpool.tile([1, 2, C8], F8)
            nc.sync.dma_start(
                bkw_sb[:], bkw.rearrange("(p i) q -> p i q", p=1))
            for ch in range(1, NCH):
                load_x(0, ch)
                load_x(1, ch)

            ident_bf = wpool.tile([128, 128], BF16)
            make_identity(nc, ident_bf[:])

            # ---- stage buffers ----
            sb192 = stage.tile([128, N_PAIR, 2 * GPC, HW], BF16)
            res = stage.tile([128, NJ, GPC], BF16)     # column c = ORDER[c]
            qk_sb = stage.tile([C8, GPC, 2, HW], F8)
            pam_sb = stage.tile([128, GPC, 2], BF16)
            kout = stage.tile([NJ, GPC, 128], BF16)
            pout = stage.tile([2 * GPC, 128], BF16)

            # ---------- QAConv helpers ----------
            a_pend = []
            a_second = set()

            def qa_mms(j, ccs):
                """Issue matmuls for j over the given cc list."""
                jc, jl = divmod(j, JPC)
                if PATHS[j] == "D":
                    # slot-major: finish each psum slot's accumulation
                    # before starting the next (interleaved start groups
                    # within one psum bank corrupt each other on HW)
                    tiles = qa_tiles[j]
                    for half in range(2):
                        ps = tiles[half]
                        for gs in range(2):
                            g = half * 2 + gs
                            for cc in ccs:
                                lhsT = xt[cc][jc][:, :,
                                                  jl * 128:(jl + 1) * 128]
                                nc.tensor.matmul(
                                    ps[:, gs, :], lhsT,
                                    xt[cc][0][:, :, g * HW:(g + 1) * HW],
                                    start=(cc == 0), stop=(cc == 1),
                                    perf_mode=DR)
                    return
                # A path: j is the first or second of a pair; moving axis
                # covered in 256-col chunks across 3 full-bank tiles
                first = j not in a_second
                pj = j if first else j - 1
                tiles = qa_tiles[pj]
                half = 0 if first else 1   # which j of the pair
                for ch in range(3):        # this j's three 256-col chunks
                    gch = half * 3 + ch
                    ps = tiles[gch // 2]
                    for cc in ccs:
                        lhsT = xt[cc][jc][:, :, jl * 128:(jl + 1) * 128]
                        nc.tensor.matmul(
                            ps[:, gch % 2, :], lhsT,
                            xt[cc][0][:, :, ch * 256:(ch + 1) * 256],
                            start=(cc == 0), stop=(cc == 1),
                            perf_mode=DR)

            def qa_reduce(j):
                path = PATHS[j]
                col = COL_OF_J[j]
                if path == "D":
                    t0, t1 = qa_tiles[j]
                    nc.vector.tensor_reduce(
                        res[:, col, 0:2, None], t0[:], op=MAX, axis=AX)
                    nc.vector.tensor_reduce(
                        res[:, col, 2:4, None], t1[:], op=MAX, axis=AX)
                    return
                if j in a_second:
                    # second j of the pair: copy the three bank tiles
                    pj = j - 1
                    tiles = qa_tiles[pj]
                    slot = len(a_pend) + sum(len(b) for b in a_batches)
                    flat = sb192[:, slot, :, :].rearrange("p g t -> p (g t)")
                    for i in range(3):
                        nc.scalar.copy(
                            flat[:, i * 512:(i + 1) * 512],
                            tiles[i][:].rearrange("p a b -> p (a b)"))
                    a_pend.append((slot, col - 1))
                    if slot + 1 in A_FLUSH:
                        flush_a()

            a_batches = []

            def flush_a():
                if not a_pend:
                    return
                batch = list(a_pend)
                a_pend.clear()
                a_batches.append(batch)
                s0 = batch[0][0]
                c0 = batch[0][1]
                k = len(batch)
                cview = sb192[:, s0:s0 + k, :, :]
                _tree_max(nc, cview, 192)
                nc.vector.tensor_reduce(
                    res[:, c0:c0 + 2 * k, :, None].rearrange(
                        "p (a b) g w -> p a (b g) w", a=k),
                    cview[:, :, :, 0:3], op=MAX, axis=AX)

            # psum tiles per j (allocated lazily, ring via tag)
            qa_tiles = {}

            def alloc_qa(j):
                if PATHS[j] == "D":
                    qa_tiles[j] = (
                        qad_psum.tile([128, 2, HW], F32, tag="qad",
                                      name=f"qa_{j}_0"),
                        qad_psum.tile([128, 2, HW], F32, tag="qad",
                                      name=f"qa_{j}_1"),
                    )
                elif j not in a_second:
                    a_second.add(j + 1)
                    qa_tiles[j] = tuple(
                        qaa_psum.tile([128, 2, 256], F32, tag="qaa",
                                      name=f"qa_{j}_{i}")
                        for i in range(3))

            # ---------- PAM projections ----------
            # bias folded in as two extra contraction channels (ones/zeros)
            def pam_proj():
                if HAS_BIAS[0]:
                    cst = wpool.tile([1, 2, HW], F8)
                    nc.vector.memset(cst[:, 0, :], 1.0)
                    nc.vector.memset(cst[:, 1, :], 0.0)
                for b in range(GPC):
                    pp = pam_psum.tile([C8, 2, HW], F32, tag="pam",
                                       name=f"proj_{b}")
                    rhs0 = xt[0][0][:, :, b * HW:(b + 1) * HW]
                    rhs1 = xt[1][0][:, :, b * HW:(b + 1) * HW]
                    for qk, w_sb, bw in ((0, wq_sb, bqw_sb), (1, wk_sb, bkw_sb)):
                        nc.tensor.matmul(pp[:, qk, :], w_sb[0][:], rhs0,
                                         start=True, stop=False, perf_mode=DR)
                        nc.tensor.matmul(pp[:, qk, :], w_sb[1][:], rhs1,
                                         start=False,
                                         stop=not HAS_BIAS[0], perf_mode=DR)
                        if HAS_BIAS[0]:
                            nc.tensor.matmul(pp[:, qk, :], bw[:], cst[:],
                                             start=False, stop=True,
                                             perf_mode=DR)
                    nc.scalar.activation(qk_sb[:, b, :, :], pp[:], COPY,
                                         bias=0.0, scale=1.0 / WSCALE)

            def pam_out():
                tpp = pam_psum.tile([2 * GPC, 128], BF16, tag="pam",
                                    name="tpp")
                nc.tensor.transpose(
                    tpp[:], pam_sb[:].rearrange("p b h -> p (b h)"),
                    ident_bf[:])
                nc.scalar.copy(pout[:], tpp[:])
                nc.sync.dma_start(pmax_bf[:], pout[:])

            def pam_energy():
                for b in range(GPC):
                    for mch, (m0, msz) in enumerate(((0, 128), (128, 64))):
                        e = pam_psum.tile([128, HW], F32, tag="pam",
                                          name=f"e_{b}_{mch}")
                        nc.tensor.matmul(
                            e[:msz, :], qk_sb[:, b, 0, m0:m0 + msz],
                            qk_sb[:, b, 1, :], start=True, stop=True)
                        nc.vector.tensor_reduce(
                            pam_sb[:msz, b, mch:mch + 1], e[:msz, :],
                            op=MAX, axis=AX)

            # ---------- schedule ----------
            # start QA j0/j1 cc0 as soon as the first x tile lands, then
            # PAM projections (need weights + both cc of ch0)
            alloc_qa(0)
            alloc_qa(1)
            qa_mms(0, [0, 1])
            pam_proj()
            qa_mms(1, [0, 1])
            qa_reduce(0)
            qa_reduce(1)
            for j in range(2, NJ):
                alloc_qa(j)
                qa_mms(j, [0, 1])
                qa_reduce(j)
                if j == 12:
                    pam_energy()
                if j == 20:
                    pam_out()
            flush_a()

            # ---------- output: transpose + copy + DMA ----------
            tpk = pam_psum.tile([NJ, GPC, 128], BF16, tag="pam", name="tpk")
            for g in range(GPC):
                nc.tensor.transpose(tpk[:, g, :], res[:, :, g], ident_bf[:])
            nc.scalar.copy(kout[:], tpk[:])
            nc.sync.dma_start(kmax_bf[:], kout[:])

    nc.compile()
    return nc


def kernel(x, Wq, bq, Wk, bk):
    global _COMPILED
    has_bias = bool(np.any(np.asarray(bq)) or np.any(np.asarray(bk)))
    if _COMPILED is None or _COMPILED[1] != has_bias:
        HAS_BIAS[0] = has_bias
        _COMPILED = (_build(), has_bias)
    nc = _COMPILED[0]

    x = np.ascontiguousarray(x, dtype=np.float32)
    xT = np.ascontiguousarray(
        x.reshape(B, C, HW).transpose(1, 0, 2).reshape(C, FLAT))
    x8 = xT.astype(NP_F8)
    wq8 = np.ascontiguousarray(
        (np.asarray(Wq, np.float32).T * WSCALE)).astype(NP_F8)
    wk8 = np.ascontiguousarray(
        (np.asarray(Wk, np.float32).T * WSCALE)).astype(NP_F8)
    bqa = np.zeros((2, C8), np.float32)
    bqa[0] = np.asarray(bq, np.float32) * WSCALE
    bka = np.zeros((2, C8), np.float32)
    bka[0] = np.asarray(bk, np.float32) * WSCALE
    bq8 = bqa.astype(NP_F8)
    bk8 = bka.astype(NP_F8)

    in_maps = [
        {
            "x8": np.ascontiguousarray(np.roll(x8, -i * GROLL, axis=1)),
            "wq8": wq8,
            "wk8": wk8,
            "bqw": bq8,
            "bkw": bk8,
        }
        for i in range(N_CORES)
    ]

    res = run_bass_kernel_spmd(nc, in_maps, core_ids=list(range(N_CORES)))

    order = np.asarray(ORDER)
    kernel_max = np.empty((B, FLAT), np.float32)
    pos_max = np.empty((B, HW), np.float32)
    for i, r in enumerate(res.results):
        km = np.asarray(r["kmax_bf"]).astype(np.float32).reshape(NJ, GPC, 128)
        km_j = np.empty_like(km)
        km_j[order] = km                        # undo path permutation
        km_flat = km_j.transpose(1, 0, 2).reshape(GPC, FLAT)
        kernel_max[i * GPC:(i + 1) * GPC] = np.roll(
            km_flat, i * GROLL, axis=1)
        pm = np.asarray(r["pmax_bf"]).astype(np.float32).reshape(2 * GPC, 128)
        for b in range(GPC):
            pos_max[i * GPC + b, 0:128] = pm[2 * b]
            pos_max[i * GPC + b, 128:HW] = pm[2 * b + 1, 0:64]

    return np.concatenate(
        [kernel_max.reshape(B, B, HW), pos_max[None]], axis=0
    ).astype(np.float32)


# revision 3
# speedup vs baseline: 1.0060x; 1.0060x over previous
"""Trainium2 Bass kernel for nn_AlignModule (QAConv correlation + PAM), fp8.

Reference computation (B=32, C=512, H=24, W=8, hw=192, C8=64):
  xf = x.reshape(B, C, hw)
  score[g,p,n,m] = sum_c xf[g,c,m] * xf[p,c,n]          # [B,B,hw,hw]
  kernel_max[g,p,n] = max_m score[g,p,n,m]              # [B,B,hw]
  q = Wq @ xf[b] + bq; k = Wk @ xf[b] + bk              # [B,C8,hw]
  energy[b,m,n] = sum_q q[b,q,m] k[b,q,n]
  pos_max[b,m] = max_n energy[b,m,n]                    # [B,hw]
  out = concat([kernel_max, pos_max[None]], axis=0)     # [B+1,B,hw]

Sharding: data-parallel over g across 8 cores (4 images each).  Each core
receives the full x as [C, B*hw] fp8e4m3, rolled so its own images occupy
columns [0, 768).

QAConv matmuls run in fp8 DoubleRow perf mode: each instruction contracts
256 channels (128 partitions x 2 double-rows) at 0.5 cycles per moving
column -- 4x the fp32r rate of the previous kernel.  The 2e-2 relative
error budget (vs output absmax ~639) covers fp8 quantization (~1.2%).

The per-block max reductions (96 psum blocks of [128, 2, 192] per core)
are split across three engines so they keep pace with the PE:
  - DVE: tensor_tensor_reduce directly from PSUM (per-g halves)
  - Pool: tensor_max (halves) PSUM -> SBUF bf16, finished by a batched
    DVE bf16 max tree (2x_1p mode) 96 -> 1
  - Act: copy PSUM -> SBUF bf16, same DVE tree from 192
Stage-2 results land in a path-permuted column order; the host unpermutes.
"""

import numpy as np
import ml_dtypes

import concourse.bass as bass
import concourse.mybir as mybir
import concourse.tile as tile
from concourse import bacc
from concourse.bass_utils import run_bass_kernel_spmd
from concourse.masks import make_identity

B = 32
C = 512
HW = 192
C8 = 64
N_CORES = 8
GPC = B // N_CORES            # images per core (4)
FLAT = B * HW                 # flattened (p, n) axis (6144)
GROLL = GPC * HW              # per-core roll step (768)
NCH = FLAT // GROLL           # column chunks of 768 (8)
NJ = FLAT // 128              # stationary 128-column blocks (48)
JPC = GROLL // 128            # j blocks per column chunk (6)
WSCALE = 64.0                 # host premultiplier for Wq/Wk before fp8 cast

F32 = mybir.dt.float32
BF16 = mybir.dt.bfloat16
F8 = mybir.dt.float8e4
AX = mybir.AxisListType.X
MAX = mybir.AluOpType.max
DR = mybir.MatmulPerfMode.DoubleRow
IDENT = mybir.ActivationFunctionType.Identity
COPY = mybir.ActivationFunctionType.Copy
HAS_BIAS = [False]  # set by kernel() before (re)build

NP_F8 = ml_dtypes.float8_e4m3
NP_BF16 = ml_dtypes.bfloat16

# reduce-path assignment over the 48 j blocks:
# D = DVE tensor_tensor_reduce direct, P = Pool tensor_max + DVE tree,
# A = Act copy + DVE tree.  D js early (fill DVE while trees wait for
# egress); A js kept away from the tail (their trees are heavy).
import os
_D_POS_BY_N = {
    12: [0, 3, 8, 11, 16, 19, 24, 27, 32, 37, 42, 47],
    14: [0, 3, 6, 11, 14, 17, 22, 25, 28, 33, 36, 39, 44, 47],
    16: [0, 3, 6, 9, 12, 15, 18, 21, 24, 27, 30, 35, 38, 41, 44, 47],
}
D_POS = _D_POS_BY_N[int(os.environ.get("K_ND", "14"))]


def _default_sched():
    # Walrus-legal reduce engines are only DVE and Act: D = DVE direct
    # tensor_reduce, A = Act copy (full-bank pairs) + batched DVE bf16
    # tree.  A js come in consecutive even-length runs (pairs share 3
    # psum banks); D placement keeps runs even and covers the tail.
    out = ["A"] * NJ
    for p in D_POS:
        out[p] = "D"
    return "".join(out)


_SCHED = os.environ.get("K_SCHED") or _default_sched()
PATHS = list(_SCHED)
assert len(PATHS) == NJ, (len(PATHS), _SCHED)
N_D = PATHS.count("D")
N_P = PATHS.count("P")
N_A = PATHS.count("A")
# stage-2 flush thresholds (cumulative slots; flush when slot count hits one)
def _flushes(n, k):
    nb = max(1, round(n / k))
    out = [round(n * (i + 1) / nb) for i in range(nb)]
    return out
N_PAIR = N_A // 2
A_FLUSH = [3, 6, 9, 12, 14, 15, 16, 17][:N_PAIR]
if A_FLUSH[-1] != N_PAIR:
    A_FLUSH = _flushes(N_PAIR, 3)

_COMPILED = None
# res column order: D js first, then P js, then A js (each in j order)
ORDER = ([j for j, p in enumerate(PATHS) if p == "D"]
         + [j for j, p in enumerate(PATHS) if p == "P"]
         + [j for j, p in enumerate(PATHS) if p == "A"])
COL_OF_J = {j: c for c, j in enumerate(ORDER)}


def _tree_max(nc, c, width):
    """In-place DVE bf16 max tree over the last axis: width -> 3."""
    w = width
    while w > 3:
        h = w // 2
        nc.vector.tensor_max(c[:, :, :, 0:h], c[:, :, :, 0:h], c[:, :, :, h:w])
        w = h


def _build():
    nc = bacc.Bacc("TRN2", target_bir_lowering=False, debug=False)

    x8 = nc.dram_tensor("x8", [C, FLAT], F8, kind="ExternalInput").ap()
    wq8 = nc.dram_tensor("wq8", [C, C8], F8, kind="ExternalInput").ap()
    wk8 = nc.dram_tensor("wk8", [C, C8], F8, kind="ExternalInput").ap()
    bqw = nc.dram_tensor("bqw", [2, C8], F8, kind="ExternalInput").ap()
    bkw = nc.dram_tensor("bkw", [2, C8], F8, kind="ExternalInput").ap()
    # kmax_bf[c, g, t]: kernel_max[g, ORDER[c]*128 + t] (rolled flat order)
    kmax_bf = nc.dram_tensor("kmax_bf", [NJ, GPC, 128], BF16,
                             kind="ExternalOutput").ap()
    # pmax_bf[2*b+h, t]: pos_max[b, h*128 + t] (h=1 valid for t < 64)
    pmax_bf = nc.dram_tensor("pmax_bf", [2 * GPC, 128], BF16,
                             kind="ExternalOutput").ap()

    with tile.TileContext(nc) as tc:
        with (
            tc.tile_pool(name="xpool", bufs=1) as xpool,
            tc.tile_pool(name="wpool", bufs=1) as wpool,
            tc.tile_pool(name="stage", bufs=1) as stage,
            tc.tile_pool(name="qad_psum", bufs=2, space="PSUM") as qad_psum,
            tc.tile_pool(name="qaa_psum", bufs=4, space="PSUM") as qaa_psum,
            tc.tile_pool(name="pam_psum", bufs=2, space="PSUM") as pam_psum,
        ):
            # ---- x tiles [128, 2, 768] fp8 per (cc, ch); channel
            # c = cc*256 + i*128 + k lives at tile[k, i, :] ----
            xt = [[None] * NCH for _ in range(2)]
            wq_sb, wk_sb = [None, None], [None, None]

            def load_x(cc, ch):
                t = xpool.tile([128, 2, GROLL], F8, tag=f"x_{cc}_{ch}", name=f"x_{cc}_{ch}")
                nc.sync.dma_start(
                    t[:],
                    x8[cc * 256:(cc + 1) * 256,
                       ch * GROLL:(ch + 1) * GROLL].rearrange(
                        "(i p) c -> p i c", p=128),
                )
                xt[cc][ch] = t

            load_x(0, 0)
            load_x(1, 0)
            for cc2 in range(2):
                wq_sb[cc2] = wpool.tile([128, 2, C8], F8, tag=f"wq_{cc2}", name=f"wq_{cc2}")
                nc.sync.dma_start(
                    wq_sb[cc2][:],
                    wq8[cc2 * 256:(cc2 + 1) * 256, :].rearrange(
                        "(i p) q -> p i q", p=128))
                wk_sb[cc2] = wpool.tile([128, 2, C8], F8, tag=f"wk_{cc2}", name=f"wk_{cc2}")
                nc.sync.dma_start(
                    wk_sb[cc2][:],
                    wk8[cc2 * 256:(cc2 + 1) * 256, :].rearrange(
                        "(i p) q -> p i q", p=128))
            bqw_sb = wpool.tile([1, 2, C8], F8)
            nc.sync.dma_start(
                bqw_sb[:], bqw.rearrange("(p i) q -> p i q", p=1))
            bkw_sb = wpool.tile([1, 2, C8], F8)
            nc.sync.dma_start(
                bkw_sb[:], bkw.rearrange("(p i) q -> p i q", p=1))
            for ch in range(1, NCH):
                load_x(0, ch)
                load_x(1, ch)

            ident_bf = wpool.tile([128, 128], BF16)
            make_identity(nc, ident_bf[:])

            # ---- stage buffers ----
            sb192 = stage.tile([128, N_PAIR, 2 * GPC, HW], BF16)
            res = stage.tile([128, NJ, GPC], BF16)     # column c = ORDER[c]
            qk_sb = stage.tile([C8, GPC, 2, HW], F8)
            pam_sb = stage.tile([128, GPC, 2], BF16)
            kout = stage.tile([NJ, GPC, 128], BF16)
            pout = stage.tile([2 * GPC, 128], BF16)

            # ---------- QAConv helpers ----------
            a_pend = []
            a_second = set()

            def qa_mms(j, ccs):
                """Issue matmuls for j over the given cc list."""
                jc, jl = divmod(j, JPC)
                if PATHS[j] == "D":
                    # slot-major: finish each psum slot's accumulation
                    # before starting the next (interleaved start groups
                    # within one psum bank corrupt each other on HW)
                    tiles = qa_tiles[j]
                    for half in range(2):
                        ps = tiles[half]
                        for gs in range(2):
                            g = half * 2 + gs
                            for cc in ccs:
                                lhsT = xt[cc][jc][:, :,
                                                  jl * 128:(jl + 1) * 128]
                                nc.tensor.matmul(
                                    ps[:, gs, :], lhsT,
                                    xt[cc][0][:, :, g * HW:(g + 1) * HW],
                                    start=(cc == 0), stop=(cc == 1),
                                    perf_mode=DR)
                    return
                # A path: j is the first or second of a pair; moving axis
                # covered in 256-col chunks across 3 full-bank tiles
                first = j not in a_second
                pj = j if first else j - 1
                tiles = qa_tiles[pj]
                half = 0 if first else 1   # which j of the pair
                for ch in range(3):        # this j's three 256-col chunks
                    gch = half * 3 + ch
                    ps = tiles[gch // 2]
                    for cc in ccs:
                        lhsT = xt[cc][jc][:, :, jl * 128:(jl + 1) * 128]
                        nc.tensor.matmul(
                            ps[:, gch % 2, :], lhsT,
                            xt[cc][0][:, :, ch * 256:(ch + 1) * 256],
                            start=(cc == 0), stop=(cc == 1),
                            perf_mode=DR)

            def qa_reduce(j):
                path = PATHS[j]
                col = COL_OF_J[j]
                if path == "D":
                    t0, t1 = qa_tiles[j]
                    nc.vector.tensor_reduce(
                        res[:, col, 0:2, None], t0[:], op=MAX, axis=AX)
                    nc.vector.tensor_reduce(
                        res[:, col, 2:4, None], t1[:], op=MAX, axis=AX)
                    return
                if j in a_second:
                    # second j of the pair: copy the three bank tiles
                    pj = j - 1
                    tiles = qa_tiles[pj]
                    slot = len(a_pend) + sum(len(b) for b in a_batches)
                    flat = sb192[:, slot, :, :].rearrange("p g t -> p (g t)")
                    for i in range(3):
                        nc.scalar.copy(
                            flat[:, i * 512:(i + 1) * 512],
                            tiles[i][:].rearrange("p a b -> p (a b)"))
                    a_pend.append((slot, col - 1))
                    if slot + 1 in A_FLUSH:
                        flush_a()

            a_batches = []

            def flush_a():
                if not a_pend:
                    return
                batch = list(a_pend)
                a_pend.clear()
                a_batches.append(batch)
                s0 = batch[0][0]
                c0 = batch[0][1]
                k = len(batch)
                cview = sb192[:, s0:s0 + k, :, :]
                _tree_max(nc, cview, 192)
                nc.vector.tensor_reduce(
                    res[:, c0:c0 + 2 * k, :, None].rearrange(
                        "p (a b) g w -> p a (b g) w", a=k),
                    cview[:, :, :, 0:3], op=MAX, axis=AX)

            # psum tiles per j (allocated lazily, ring via tag)
            qa_tiles = {}

            def alloc_qa(j):
                if PATHS[j] == "D":
                    qa_tiles[j] = (
                        qad_psum.tile([128, 2, HW], F32, tag="qad",
                                      name=f"qa_{j}_0"),
                        qad_psum.tile([128, 2, HW], F32, tag="qad",
                                      name=f"qa_{j}_1"),
                    )
                elif j not in a_second:
                    a_second.add(j + 1)
                    qa_tiles[j] = tuple(
                        qaa_psum.tile([128, 2, 256], F32, tag="qaa",
                                      name=f"qa_{j}_{i}")
                        for i in range(3))

            # ---------- PAM projections ----------
            # bias folded in as two extra contraction channels (ones/zeros)
            def pam_proj():
                if HAS_BIAS[0]:
                    cst = wpool.tile([1, 2, HW], F8)
                    nc.vector.memset(cst[:, 0, :], 1.0)
                    nc.vector.memset(cst[:, 1, :], 0.0)
                for b in range(GPC):
                    pp = pam_psum.tile([C8, 2, HW], F32, tag="pam",
                                       name=f"proj_{b}")
                    rhs0 = xt[0][0][:, :, b * HW:(b + 1) * HW]
                    rhs1 = xt[1][0][:, :, b * HW:(b + 1) * HW]
                    for qk, w_sb, bw in ((0, wq_sb, bqw_sb), (1, wk_sb, bkw_sb)):
                        nc.tensor.matmul(pp[:, qk, :], w_sb[0][:], rhs0,
                                         start=True, stop=False, perf_mode=DR)
                        nc.tensor.matmul(pp[:, qk, :], w_sb[1][:], rhs1,
                                         start=False,
                                         stop=not HAS_BIAS[0], perf_mode=DR)
                        if HAS_BIAS[0]:
                            nc.tensor.matmul(pp[:, qk, :], bw[:], cst[:],
                                             start=False, stop=True,
                                             perf_mode=DR)
                    nc.scalar.activation(qk_sb[:, b, :, :], pp[:], COPY,
                                         bias=0.0, scale=1.0 / WSCALE)

            def pam_out():
                tpp = pam_psum.tile([2 * GPC, 128], BF16, tag="pam",
                                    name="tpp")
                nc.tensor.transpose(
                    tpp[:], pam_sb[:].rearrange("p b h -> p (b h)"),
                    ident_bf[:])
                nc.scalar.copy(pout[:], tpp[:])
                nc.sync.dma_start(pmax_bf[:], pout[:])

            def pam_energy():
                for b in range(GPC):
                    for mch, (m0, msz) in enumerate(((0, 128), (128, 64))):
                        e = pam_psum.tile([128, HW], F32, tag="pam",
                                          name=f"e_{b}_{mch}")
                        nc.tensor.matmul(
                            e[:msz, :], qk_sb[:, b, 0, m0:m0 + msz],
                            qk_sb[:, b, 1, :], start=True, stop=True)
                        nc.vector.tensor_reduce(
                            pam_sb[:msz, b, mch:mch + 1], e[:msz, :],
                            op=MAX, axis=AX)

            # ---------- schedule ----------
            # start QA j0/j1 cc0 as soon as the first x tile lands, then
            # PAM projections (need weights + both cc of ch0)
            alloc_qa(0)
            alloc_qa(1)
            qa_mms(0, [0, 1])
            pam_proj()
            qa_mms(1, [0, 1])
            qa_reduce(0)
            qa_reduce(1)
            for j in range(2, NJ):
                alloc_qa(j)
                qa_mms(j, [0, 1])
                qa_reduce(j)
                if j == 12:
                    pam_energy()
                if j == 20:
                    pam_out()
            flush_a()

            # ---------- output: transpose + copy + DMA ----------
            tpk = pam_psum.tile([NJ, GPC, 128], BF16, tag="pam", name="tpk")
            for g in range(GPC):
                nc.tensor.transpose(tpk[:, g, :], res[:, :, g], ident_bf[:])
            nc.vector.tensor_copy(kout[:], tpk[:])
            nc.sync.dma_start(kmax_bf[:], kout[:])

    nc.compile()
    return nc


def kernel(x, Wq, bq, Wk, bk):
    global _COMPILED
    has_bias = bool(np.any(np.asarray(bq)) or np.any(np.asarray(bk)))
    if _COMPILED is None or _COMPILED[1] != has_bias:
        HAS_BIAS[0] = has_bias
        _COMPILED = (_build(), has_bias)
    nc = _COMPILED[0]

    x = np.ascontiguousarray(x, dtype=np.float32)
    xT = np.ascontiguousarray(
        x.reshape(B, C, HW).transpose(1, 0, 2).reshape(C, FLAT))
    x8 = xT.astype(NP_F8)
    wq8 = np.ascontiguousarray(
        (np.asarray(Wq, np.float32).T * WSCALE)).astype(NP_F8)
    wk8 = np.ascontiguousarray(
        (np.asarray(Wk, np.float32).T * WSCALE)).astype(NP_F8)
    bqa = np.zeros((2, C8), np.float32)
    bqa[0] = np.asarray(bq, np.float32) * WSCALE
    bka = np.zeros((2, C8), np.float32)
    bka[0] = np.asarray(bk, np.float32) * WSCALE
    bq8 = bqa.astype(NP_F8)
    bk8 = bka.astype(NP_F8)

    in_maps = [
        {
            "x8": np.ascontiguousarray(np.roll(x8, -i * GROLL, axis=1)),
            "wq8": wq8,
            "wk8": wk8,
            "bqw": bq8,
            "bkw": bk8,
        }
        for i in range(N_CORES)
    ]

    res = run_bass_kernel_spmd(nc, in_maps, core_ids=list(range(N_CORES)))

    order = np.asarray(ORDER)
    kernel_max = np.empty((B, FLAT), np.float32)
    pos_max = np.empty((B, HW), np.float32)
    for i, r in enumerate(res.results):
        km = np.asarray(r["kmax_bf"]).astype(np.float32).reshape(NJ, GPC, 128)
        km_j = np.empty_like(km)
        km_j[order] = km                        # undo path permutation
        km_flat = km_j.transpose(1, 0, 2).reshape(GPC, FLAT)
        kernel_max[i * GPC:(i + 1) * GPC] = np.roll(
            km_flat, i * GROLL, axis=1)
        pm = np.asarray(r["pmax_bf"]).astype(np.float32).reshape(2 * GPC, 128)
        for b in range(GPC):
            pos_max[i * GPC + b, 0:128] = pm[2 * b]
            pos_max[i * GPC + b, 128:HW] = pm[2 * b + 1, 0:64]

    return np.concatenate(
        [kernel_max.reshape(B, B, HW), pos_max[None]], axis=0
    ).astype(np.float32)


# revision 5
# speedup vs baseline: 1.0341x; 1.0280x over previous
"""Trainium2 Bass kernel for nn_AlignModule (QAConv correlation + PAM), fp8.

Reference computation (B=32, C=512, H=24, W=8, hw=192, C8=64):
  xf = x.reshape(B, C, hw)
  score[g,p,n,m] = sum_c xf[g,c,m] * xf[p,c,n]          # [B,B,hw,hw]
  kernel_max[g,p,n] = max_m score[g,p,n,m]              # [B,B,hw]
  q = Wq @ xf[b] + bq; k = Wk @ xf[b] + bk              # [B,C8,hw]
  energy[b,m,n] = sum_q q[b,q,m] k[b,q,n]
  pos_max[b,m] = max_n energy[b,m,n]                    # [B,hw]
  out = concat([kernel_max, pos_max[None]], axis=0)     # [B+1,B,hw]

Sharding: data-parallel over g across 8 cores (4 images each).  Each core
receives the full x as [C, B*hw] fp8e4m3, rolled so its own images occupy
columns [0, 768).

QAConv matmuls run in fp8 DoubleRow perf mode: each instruction contracts
256 channels (128 partitions x 2 double-rows) at 0.5 cycles per moving
column -- 4x the fp32r rate of the previous kernel.  The 2e-2 relative
error budget (vs output absmax ~639) covers fp8 quantization (~1.2%).

The per-block max reductions (4.7M psum floats per core) are bound by
hard TRN2 rules: GPSIMD cannot touch PSUM, DMA cannot read PSUM, and no
instruction may read two non-scalar PSUM inputs.  That leaves two legal
egress paths, balanced ~1:2 so DVE and Act both run ~34us:
  - D js: DVE tensor_reduce straight from PSUM into bf16 results
  - A js (in pairs): Act copies three full-bank [128,2,256] psum tiles
    to an SBUF bf16 slab (full-bank tiles amortize Act's fixed access
    cost and use 3 banks per 2 js); a batched in-place DVE tensor_max
    tree (bf16 2x_1p mode) then reduces 192 -> 1
PSUM accumulation groups are issued slot-major: interleaving start=True
groups within one psum bank corrupts earlier partial sums on hardware.
Results land in a path-permuted column order; the host unpermutes.
"""

import numpy as np
import ml_dtypes

import concourse.bass as bass
import concourse.mybir as mybir
import concourse.tile as tile
from concourse import bacc
from concourse.bass_utils import run_bass_kernel_spmd
from concourse.masks import make_identity

B = 32
C = 512
HW = 192
C8 = 64
N_CORES = 8
GPC = B // N_CORES            # images per core (4)
FLAT = B * HW                 # flattened (p, n) axis (6144)
GROLL = GPC * HW              # per-core roll step (768)
NCH = FLAT // GROLL           # column chunks of 768 (8)
NJ = FLAT // 128              # stationary 128-column blocks (48)
JPC = GROLL // 128            # j blocks per column chunk (6)
WSCALE = 64.0                 # host premultiplier for Wq/Wk before fp8 cast

F32 = mybir.dt.float32
BF16 = mybir.dt.bfloat16
F8 = mybir.dt.float8e4
AX = mybir.AxisListType.X
MAX = mybir.AluOpType.max
DR = mybir.MatmulPerfMode.DoubleRow
IDENT = mybir.ActivationFunctionType.Identity
COPY = mybir.ActivationFunctionType.Copy
HAS_BIAS = [False]  # set by kernel() before (re)build

NP_F8 = ml_dtypes.float8_e4m3
NP_BF16 = ml_dtypes.bfloat16

# reduce-path assignment over the 48 j blocks:
# D = DVE tensor_tensor_reduce direct, P = Pool tensor_max + DVE tree,
# A = Act copy + DVE tree.  D js early (fill DVE while trees wait for
# egress); A js kept away from the tail (their trees are heavy).
import os
_D_POS_BY_N = {
    12: [0, 3, 8, 11, 16, 19, 24, 27, 32, 37, 42, 47],
    14: [0, 3, 6, 11, 14, 17, 22, 25, 28, 33, 36, 39, 44, 47],
    16: [0, 3, 6, 9, 12, 15, 18, 21, 24, 27, 30, 35, 38, 41, 44, 47],
}
D_POS = _D_POS_BY_N[int(os.environ.get("K_ND", "14"))]


def _default_sched():
    # Walrus-legal reduce engines are only DVE and Act: D = DVE direct
    # tensor_reduce, A = Act copy (full-bank pairs) + batched DVE bf16
    # tree.  A js come in consecutive even-length runs (pairs share 3
    # psum banks); D placement keeps runs even and covers the tail.
    out = ["A"] * NJ
    for p in D_POS:
        out[p] = "D"
    return "".join(out)


_SCHED = os.environ.get("K_SCHED") or _default_sched()
PATHS = list(_SCHED)
assert len(PATHS) == NJ, (len(PATHS), _SCHED)
N_D = PATHS.count("D")
N_P = PATHS.count("P")
N_A = PATHS.count("A")
# stage-2 flush thresholds (cumulative slots; flush when slot count hits one)
def _flushes(n, k):
    nb = max(1, round(n / k))
    out = [round(n * (i + 1) / nb) for i in range(nb)]
    return out
N_PAIR = N_A // 2
A_FLUSH = [3, 6, 9, 12, 14, 15, 16, 17][:N_PAIR]
if A_FLUSH[-1] != N_PAIR:
    A_FLUSH = _flushes(N_PAIR, 3)

_COMPILED = None
# res column order: D js first, then P js, then A js (each in j order)
ORDER = ([j for j, p in enumerate(PATHS) if p == "D"]
         + [j for j, p in enumerate(PATHS) if p == "P"]
         + [j for j, p in enumerate(PATHS) if p == "A"])
COL_OF_J = {j: c for c, j in enumerate(ORDER)}


def _tree_max(nc, c, width):
    """In-place DVE bf16 max tree over the last axis: width -> 3."""
    w = width
    while w > 3:
        h = w // 2
        nc.vector.tensor_max(c[:, :, :, 0:h], c[:, :, :, 0:h], c[:, :, :, h:w])
        w = h


def _build():
    nc = bacc.Bacc("TRN2", target_bir_lowering=False, debug=False)

    x8 = nc.dram_tensor("x8", [C, FLAT], F8, kind="ExternalInput").ap()
    wq8 = nc.dram_tensor("wq8", [C, C8], F8, kind="ExternalInput").ap()
    wk8 = nc.dram_tensor("wk8", [C, C8], F8, kind="ExternalInput").ap()
    bqw = nc.dram_tensor("bqw", [2, C8], F8, kind="ExternalInput").ap()
    bkw = nc.dram_tensor("bkw", [2, C8], F8, kind="ExternalInput").ap()
    # kmax_bf[c, g, t]: kernel_max[g, ORDER[c]*128 + t] (rolled flat order)
    kmax_bf = nc.dram_tensor("kmax_bf", [NJ, GPC, 128], BF16,
                             kind="ExternalOutput").ap()
    # pmax_bf[2*b+h, t]: pos_max[b, h*128 + t] (h=1 valid for t < 64)
    pmax_bf = nc.dram_tensor("pmax_bf", [2 * GPC, 128], BF16,
                             kind="ExternalOutput").ap()

    with tile.TileContext(nc) as tc:
        with (
            tc.tile_pool(name="xpool", bufs=1) as xpool,
            tc.tile_pool(name="wpool", bufs=1) as wpool,
            tc.tile_pool(name="stage", bufs=1) as stage,
            tc.tile_pool(name="qad_psum", bufs=2, space="PSUM") as qad_psum,
            tc.tile_pool(name="qaa_psum", bufs=4, space="PSUM") as qaa_psum,
            tc.tile_pool(name="pam_psum", bufs=2, space="PSUM") as pam_psum,
        ):
            # ---- x tiles [128, 2, 768] fp8 per (cc, ch); channel
            # c = cc*256 + i*128 + k lives at tile[k, i, :] ----
            xt = [[None] * NCH for _ in range(2)]
            wq_sb, wk_sb = [None, None], [None, None]

            def load_x(cc, ch):
                t = xpool.tile([128, 2, GROLL], F8, tag=f"x_{cc}_{ch}", name=f"x_{cc}_{ch}")
                nc.sync.dma_start(
                    t[:],
                    x8[cc * 256:(cc + 1) * 256,
                       ch * GROLL:(ch + 1) * GROLL].rearrange(
                        "(i p) c -> p i c", p=128),
                )
                xt[cc][ch] = t

            load_x(0, 0)
            load_x(1, 0)
            for cc2 in range(2):
                wq_sb[cc2] = wpool.tile([128, 2, C8], F8, tag=f"wq_{cc2}", name=f"wq_{cc2}")
                nc.sync.dma_start(
                    wq_sb[cc2][:],
                    wq8[cc2 * 256:(cc2 + 1) * 256, :].rearrange(
                        "(i p) q -> p i q", p=128))
                wk_sb[cc2] = wpool.tile([128, 2, C8], F8, tag=f"wk_{cc2}", name=f"wk_{cc2}")
                nc.sync.dma_start(
                    wk_sb[cc2][:],
                    wk8[cc2 * 256:(cc2 + 1) * 256, :].rearrange(
                        "(i p) q -> p i q", p=128))
            bqw_sb = wpool.tile([1, 2, C8], F8)
            nc.sync.dma_start(
                bqw_sb[:], bqw.rearrange("(p i) q -> p i q", p=1))
            bkw_sb = wpool.tile([1, 2, C8], F8)
            nc.sync.dma_start(
                bkw_sb[:], bkw.rearrange("(p i) q -> p i q", p=1))
            for ch in range(1, NCH):
                load_x(0, ch)
                load_x(1, ch)

            ident_bf = wpool.tile([128, 128], BF16)
            make_identity(nc, ident_bf[:])

            # ---- stage buffers ----
            sb192 = stage.tile([128, N_PAIR, 2 * GPC, HW], BF16)
            res = stage.tile([128, NJ, GPC], BF16)     # column c = ORDER[c]
            qk_sb = stage.tile([C8, GPC, 2, HW], F8)
            pam_sb = stage.tile([128, GPC, 2], BF16)
            kout = stage.tile([NJ, GPC, 128], BF16)
            pout = stage.tile([2 * GPC, 128], BF16)

            # ---------- QAConv helpers ----------
            a_pend = []
            a_second = set()

            def qa_mms(j, ccs):
                """Issue matmuls for j over the given cc list."""
                jc, jl = divmod(j, JPC)
                if PATHS[j] == "D":
                    # slot-major: finish each psum slot's accumulation
                    # before starting the next (interleaved start groups
                    # within one psum bank corrupt each other on HW)
                    tiles = qa_tiles[j]
                    for half in range(2):
                        ps = tiles[half]
                        for gs in range(2):
                            g = half * 2 + gs
                            for cc in ccs:
                                lhsT = xt[cc][jc][:, :,
                                                  jl * 128:(jl + 1) * 128]
                                nc.tensor.matmul(
                                    ps[:, gs, :], lhsT,
                                    xt[cc][0][:, :, g * HW:(g + 1) * HW],
                                    start=(cc == 0), stop=(cc == 1),
                                    perf_mode=DR)
                    return
                # A path: j is the first or second of a pair; moving axis
                # covered in 256-col chunks across 3 full-bank tiles
                first = j not in a_second
                pj = j if first else j - 1
                tiles = qa_tiles[pj]
                half = 0 if first else 1   # which j of the pair
                for ch in range(3):        # this j's three 256-col chunks
                    gch = half * 3 + ch
                    ps = tiles[gch // 2]
                    for cc in ccs:
                        lhsT = xt[cc][jc][:, :, jl * 128:(jl + 1) * 128]
                        nc.tensor.matmul(
                            ps[:, gch % 2, :], lhsT,
                            xt[cc][0][:, :, ch * 256:(ch + 1) * 256],
                            start=(cc == 0), stop=(cc == 1),
                            perf_mode=DR)

            def qa_reduce(j):
                path = PATHS[j]
                col = COL_OF_J[j]
                if path == "D":
                    t0, t1 = qa_tiles[j]
                    nc.vector.tensor_reduce(
                        res[:, col, 0:2, None], t0[:], op=MAX, axis=AX)
                    nc.vector.tensor_reduce(
                        res[:, col, 2:4, None], t1[:], op=MAX, axis=AX)
                    return
                if j in a_second:
                    # second j of the pair: copy the three bank tiles
                    pj = j - 1
                    tiles = qa_tiles[pj]
                    slot = len(a_pend) + sum(len(b) for b in a_batches)
                    flat = sb192[:, slot, :, :].rearrange("p g t -> p (g t)")
                    for i in range(3):
                        nc.scalar.copy(
                            flat[:, i * 512:(i + 1) * 512],
                            tiles[i][:].rearrange("p a b -> p (a b)"))
                    a_pend.append((slot, col - 1))
                    if slot + 1 in A_FLUSH:
                        flush_a()

            a_batches = []

            def flush_a():
                if not a_pend:
                    return
                batch = list(a_pend)
                a_pend.clear()
                a_batches.append(batch)
                s0 = batch[0][0]
                c0 = batch[0][1]
                k = len(batch)
                cview = sb192[:, s0:s0 + k, :, :]
                _tree_max(nc, cview, 192)
                nc.vector.tensor_reduce(
                    res[:, c0:c0 + 2 * k, :, None].rearrange(
                        "p (a b) g w -> p a (b g) w", a=k),
                    cview[:, :, :, 0:3], op=MAX, axis=AX)

            # psum tiles per j (allocated lazily, ring via tag)
            qa_tiles = {}

            def alloc_qa(j):
                if PATHS[j] == "D":
                    qa_tiles[j] = (
                        qad_psum.tile([128, 2, HW], F32, tag="qad",
                                      name=f"qa_{j}_0"),
                        qad_psum.tile([128, 2, HW], F32, tag="qad",
                                      name=f"qa_{j}_1"),
                    )
                elif j not in a_second:
                    a_second.add(j + 1)
                    qa_tiles[j] = tuple(
                        qaa_psum.tile([128, 2, 256], F32, tag="qaa",
                                      name=f"qa_{j}_{i}")
                        for i in range(3))

            # ---------- PAM projections ----------
            # bias folded in as two extra contraction channels (ones/zeros)
            def pam_proj():
                if HAS_BIAS[0]:
                    cst = wpool.tile([1, 2, HW], F8)
                    nc.vector.memset(cst[:, 0, :], 1.0)
                    nc.vector.memset(cst[:, 1, :], 0.0)
                for b in range(GPC):
                    pp = pam_psum.tile([C8, 2, HW], F32, tag="pam",
                                       name=f"proj_{b}")
                    rhs0 = xt[0][0][:, :, b * HW:(b + 1) * HW]
                    rhs1 = xt[1][0][:, :, b * HW:(b + 1) * HW]
                    for qk, w_sb, bw in ((0, wq_sb, bqw_sb), (1, wk_sb, bkw_sb)):
                        nc.tensor.matmul(pp[:, qk, :], w_sb[0][:], rhs0,
                                         start=True, stop=False, perf_mode=DR)
                        nc.tensor.matmul(pp[:, qk, :], w_sb[1][:], rhs1,
                                         start=False,
                                         stop=not HAS_BIAS[0], perf_mode=DR)
                        if HAS_BIAS[0]:
                            nc.tensor.matmul(pp[:, qk, :], bw[:], cst[:],
                                             start=False, stop=True,
                                             perf_mode=DR)
                    if b < 4:
                        nc.vector.tensor_scalar_mul(
                            qk_sb[:, b, :, :], pp[:], 1.0 / WSCALE)
                    else:
                        nc.scalar.activation(qk_sb[:, b, :, :], pp[:], COPY,
                                             bias=0.0, scale=1.0 / WSCALE)

            def pam_out():
                tpp = pam_psum.tile([2 * GPC, 128], BF16, tag="pam",
                                    name="tpp")
                nc.tensor.transpose(
                    tpp[:], pam_sb[:].rearrange("p b h -> p (b h)"),
                    ident_bf[:])
                nc.scalar.copy(pout[:], tpp[:])
                nc.sync.dma_start(pmax_bf[:], pout[:])

            def pam_energy():
                for b in range(GPC):
                    for mch, (m0, msz) in enumerate(((0, 128), (128, 64))):
                        e = pam_psum.tile([128, HW], F32, tag="pam",
                                          name=f"e_{b}_{mch}")
                        nc.tensor.matmul(
                            e[:msz, :], qk_sb[:, b, 0, m0:m0 + msz],
                            qk_sb[:, b, 1, :], start=True, stop=True)
                        nc.vector.tensor_reduce(
                            pam_sb[:msz, b, mch:mch + 1], e[:msz, :],
                            op=MAX, axis=AX)

            # ---------- schedule ----------
            # start QA j0/j1 cc0 as soon as the first x tile lands, then
            # PAM projections (need weights + both cc of ch0)
            alloc_qa(0)
            alloc_qa(1)
            qa_mms(0, [0, 1])
            pam_proj()
            qa_mms(1, [0, 1])
            qa_reduce(0)
            qa_reduce(1)
            for j in range(2, NJ):
                alloc_qa(j)
                qa_mms(j, [0, 1])
                qa_reduce(j)
                if j == 12:
                    pam_energy()
                if j == 20:
                    pam_out()
            flush_a()

            # ---------- output: transpose + copy + DMA ----------
            tpk = pam_psum.tile([NJ, GPC, 128], BF16, tag="pam", name="tpk")
            for g in range(GPC):
                nc.tensor.transpose(tpk[:, g, :], res[:, :, g], ident_bf[:])
            nc.vector.tensor_copy(kout[:], tpk[:])
            nc.sync.dma_start(kmax_bf[:], kout[:])

    nc.compile()
    return nc


def kernel(x, Wq, bq, Wk, bk):
    global _COMPILED
    has_bias = bool(np.any(np.asarray(bq)) or np.any(np.asarray(bk)))
    if _COMPILED is None or _COMPILED[1] != has_bias:
        HAS_BIAS[0] = has_bias
        _COMPILED = (_build(), has_bias)
    nc = _COMPILED[0]

    x = np.ascontiguousarray(x, dtype=np.float32)
    xT = np.ascontiguousarray(
        x.reshape(B, C, HW).transpose(1, 0, 2).reshape(C, FLAT))
    x8 = xT.astype(NP_F8)
    wq8 = np.ascontiguousarray(
        (np.asarray(Wq, np.float32).T * WSCALE)).astype(NP_F8)
    wk8 = np.ascontiguousarray(
        (np.asarray(Wk, np.float32).T * WSCALE)).astype(NP_F8)
    bqa = np.zeros((2, C8), np.float32)
    bqa[0] = np.asarray(bq, np.float32) * WSCALE
    bka = np.zeros((2, C8), np.float32)
    bka[0] = np.asarray(bk, np.float32) * WSCALE
    bq8 = bqa.astype(NP_F8)
    bk8 = bka.astype(NP_F8)

    in_maps = [
        {
            "x8": np.ascontiguousarray(np.roll(x8, -i * GROLL, axis=1)),
            "wq8": wq8,
            "wk8": wk8,
            "bqw": bq8,
            "bkw": bk8,
        }
        for i in range(N_CORES)
    ]

    res = run_bass_kernel_spmd(nc, in_maps, core_ids=list(range(N_CORES)))

    order = np.asarray(ORDER)
    kernel_max = np.empty((B, FLAT), np.float32)
    pos_max = np.empty((B, HW), np.float32)
    for i, r in enumerate(res.results):
        km = np.asarray(r["kmax_bf"]).astype(np.float32).reshape(NJ, GPC, 128)
        km_j = np.empty_like(km)
        km_j[order] = km                        # undo path permutation
        km_flat = km_j.transpose(1, 0, 2).reshape(GPC, FLAT)
        kernel_max[i * GPC:(i + 1) * GPC] = np.roll(
            km_flat, i * GROLL, axis=1)
        pm = np.asarray(r["pmax_bf"]).astype(np.float32).reshape(2 * GPC, 128)
        for b in range(GPC):
            pos_max[i * GPC + b, 0:128] = pm[2 * b]
            pos_max[i * GPC + b, 128:HW] = pm[2 * b + 1, 0:64]

    return np.concatenate(
        [kernel_max.reshape(B, B, HW), pos_max[None]], axis=0
    ).astype(np.float32)


# revision 8
# speedup vs baseline: 1.0430x; 1.0086x over previous
"""Trainium2 Bass kernel for nn_AlignModule (QAConv correlation + PAM), fp8.

Reference computation (B=32, C=512, H=24, W=8, hw=192, C8=64):
  xf = x.reshape(B, C, hw)
  score[g,p,n,m] = sum_c xf[g,c,m] * xf[p,c,n]          # [B,B,hw,hw]
  kernel_max[g,p,n] = max_m score[g,p,n,m]              # [B,B,hw]
  q = Wq @ xf[b] + bq; k = Wk @ xf[b] + bk              # [B,C8,hw]
  energy[b,m,n] = sum_q q[b,q,m] k[b,q,n]
  pos_max[b,m] = max_n energy[b,m,n]                    # [B,hw]
  out = concat([kernel_max, pos_max[None]], axis=0)     # [B+1,B,hw]

Sharding: data-parallel over g across 8 cores (4 images each).  Each core
receives the full x as [C, B*hw] fp8e4m3, rolled so its own images occupy
columns [0, 768).

QAConv matmuls run in fp8 DoubleRow perf mode: each instruction contracts
256 channels (128 partitions x 2 double-rows) at 0.5 cycles per moving
column -- 4x the fp32r rate of the previous kernel.  The 2e-2 relative
error budget (vs output absmax ~639) covers fp8 quantization (~1.2%).

The per-block max reductions (4.7M psum floats per core) are bound by
hard TRN2 rules: GPSIMD cannot touch PSUM, DMA cannot read PSUM, and no
instruction may read two non-scalar PSUM inputs.  That leaves two legal
egress paths, balanced ~1:2 so DVE and Act both run ~34us:
  - D js: DVE tensor_reduce straight from PSUM into bf16 results
  - A js (in pairs): Act copies three full-bank [128,2,256] psum tiles
    to an SBUF bf16 slab (full-bank tiles amortize Act's fixed access
    cost and use 3 banks per 2 js); a batched in-place DVE tensor_max
    tree (bf16 2x_1p mode) then reduces 192 -> 1
PSUM accumulation groups are issued slot-major: interleaving start=True
groups within one psum bank corrupts earlier partial sums on hardware.
Results land in a path-permuted column order; the host unpermutes.
"""

import numpy as np
import ml_dtypes

import concourse.bass as bass
import concourse.mybir as mybir
import concourse.tile as tile
from concourse import bacc
from concourse.bass_utils import run_bass_kernel_spmd
from concourse.masks import make_identity

B = 32
C = 512
HW = 192
C8 = 64
N_CORES = 8
GPC = B // N_CORES            # images per core (4)
FLAT = B * HW                 # flattened (p, n) axis (6144)
GROLL = GPC * HW              # per-core roll step (768)
NCH = FLAT // GROLL           # column chunks of 768 (8)
NJ = FLAT // 128              # stationary 128-column blocks (48)
JPC = GROLL // 128            # j blocks per column chunk (6)
WSCALE = 64.0                 # host premultiplier for Wq/Wk before fp8 cast

F32 = mybir.dt.float32
BF16 = mybir.dt.bfloat16
F8 = mybir.dt.float8e4
AX = mybir.AxisListType.X
MAX = mybir.AluOpType.max
DR = mybir.MatmulPerfMode.DoubleRow
IDENT = mybir.ActivationFunctionType.Identity
COPY = mybir.ActivationFunctionType.Copy
HAS_BIAS = [False]  # set by kernel() before (re)build

NP_F8 = ml_dtypes.float8_e4m3
NP_BF16 = ml_dtypes.bfloat16

# reduce-path assignment over the 48 j blocks:
# D = DVE tensor_tensor_reduce direct, P = Pool tensor_max + DVE tree,
# A = Act copy + DVE tree.  D js early (fill DVE while trees wait for
# egress); A js kept away from the tail (their trees are heavy).
import os
_D_POS_BY_N = {
    12: [0, 3, 8, 11, 16, 19, 24, 27, 32, 37, 42, 47],
    14: [0, 3, 6, 11, 14, 17, 22, 25, 28, 33, 36, 39, 44, 47],
    16: [0, 3, 6, 9, 12, 15, 18, 21, 24, 27, 30, 35, 38, 41, 44, 47],
}
D_POS = _D_POS_BY_N[int(os.environ.get("K_ND", "14"))]


def _default_sched():
    # Walrus-legal reduce engines are only DVE and Act: D = DVE direct
    # tensor_reduce, A = Act copy (full-bank pairs) + batched DVE bf16
    # tree.  A js come in consecutive even-length runs (pairs share 3
    # psum banks); D placement keeps runs even and covers the tail.
    out = ["A"] * NJ
    for p in D_POS:
        out[p] = "D"
    return "".join(out)


_BEST_SCHED = "DDAAAAAADAAAADDAAAAAAAADDDDDDAAAAAADAAAAAADAAAAD"
_SCHED = os.environ.get("K_SCHED") or _BEST_SCHED
PATHS = list(_SCHED)
assert len(PATHS) == NJ, (len(PATHS), _SCHED)
N_D = PATHS.count("D")
N_P = PATHS.count("P")
N_A = PATHS.count("A")
# stage-2 flush thresholds (cumulative slots; flush when slot count hits one)
def _flushes(n, k):
    nb = max(1, round(n / k))
    out = [round(n * (i + 1) / nb) for i in range(nb)]
    return out
N_PAIR = N_A // 2
A_FLUSH = [3, 6, 10, 15, 17][:N_PAIR]
if A_FLUSH[-1] != N_PAIR:
    A_FLUSH = _flushes(N_PAIR, 3)

_COMPILED = None
# res column order: D js first, then P js, then A js (each in j order)
ORDER = ([j for j, p in enumerate(PATHS) if p == "D"]
         + [j for j, p in enumerate(PATHS) if p == "P"]
         + [j for j, p in enumerate(PATHS) if p == "A"])
COL_OF_J = {j: c for c, j in enumerate(ORDER)}


def _tree_max(nc, c, width):
    """In-place DVE bf16 max tree over the last axis: width -> 3."""
    w = width
    while w > 3:
        h = w // 2
        nc.vector.tensor_max(c[:, :, :, 0:h], c[:, :, :, 0:h], c[:, :, :, h:w])
        w = h


def _build():
    nc = bacc.Bacc("TRN2", target_bir_lowering=False, debug=False)

    x8 = nc.dram_tensor("x8", [C, FLAT], F8, kind="ExternalInput").ap()
    wq8 = nc.dram_tensor("wq8", [C, C8], F8, kind="ExternalInput").ap()
    wk8 = nc.dram_tensor("wk8", [C, C8], F8, kind="ExternalInput").ap()
    bqw = nc.dram_tensor("bqw", [2, C8], F8, kind="ExternalInput").ap()
    bkw = nc.dram_tensor("bkw", [2, C8], F8, kind="ExternalInput").ap()
    # kmax_bf[c, g, t]: kernel_max[g, ORDER[c]*128 + t] (rolled flat order)
    kmax_bf = nc.dram_tensor("kmax_bf", [NJ, GPC, 128], BF16,
                             kind="ExternalOutput").ap()
    # pmax_bf[2*b+h, t]: pos_max[b, h*128 + t] (h=1 valid for t < 64)
    pmax_bf = nc.dram_tensor("pmax_bf", [2 * GPC, 128], BF16,
                             kind="ExternalOutput").ap()

    with tile.TileContext(nc) as tc:
        with (
            tc.tile_pool(name="xpool", bufs=1) as xpool,
            tc.tile_pool(name="wpool", bufs=1) as wpool,
            tc.tile_pool(name="stage", bufs=1) as stage,
            tc.tile_pool(name="qad_psum", bufs=2, space="PSUM") as qad_psum,
            tc.tile_pool(name="qaa_psum", bufs=4, space="PSUM") as qaa_psum,
            tc.tile_pool(name="pam_psum", bufs=2, space="PSUM") as pam_psum,
        ):
            # ---- x tiles [128, 2, 768] fp8 per (cc, ch); channel
            # c = cc*256 + i*128 + k lives at tile[k, i, :] ----
            xt = [[None] * NCH for _ in range(2)]
            wq_sb, wk_sb = [None, None], [None, None]

            def load_x(cc, ch):
                t = xpool.tile([128, 2, GROLL], F8, tag=f"x_{cc}_{ch}", name=f"x_{cc}_{ch}")
                nc.sync.dma_start(
                    t[:],
                    x8[cc * 256:(cc + 1) * 256,
                       ch * GROLL:(ch + 1) * GROLL].rearrange(
                        "(i p) c -> p i c", p=128),
                )
                xt[cc][ch] = t

            load_x(0, 0)
            load_x(1, 0)
            for cc2 in range(2):
                wq_sb[cc2] = wpool.tile([128, 2, C8], F8, tag=f"wq_{cc2}", name=f"wq_{cc2}")
                nc.sync.dma_start(
                    wq_sb[cc2][:],
                    wq8[cc2 * 256:(cc2 + 1) * 256, :].rearrange(
                        "(i p) q -> p i q", p=128))
                wk_sb[cc2] = wpool.tile([128, 2, C8], F8, tag=f"wk_{cc2}", name=f"wk_{cc2}")
                nc.sync.dma_start(
                    wk_sb[cc2][:],
                    wk8[cc2 * 256:(cc2 + 1) * 256, :].rearrange(
                        "(i p) q -> p i q", p=128))
            bqw_sb = wpool.tile([1, 2, C8], F8)
            nc.sync.dma_start(
                bqw_sb[:], bqw.rearrange("(p i) q -> p i q", p=1))
            bkw_sb = wpool.tile([1, 2, C8], F8)
            nc.sync.dma_start(
                bkw_sb[:], bkw.rearrange("(p i) q -> p i q", p=1))
            for ch in range(1, NCH):
                load_x(0, ch)
                load_x(1, ch)

            ident_bf = wpool.tile([128, 128], BF16)
            make_identity(nc, ident_bf[:])

            # ---- stage buffers ----
            sb192 = stage.tile([128, N_PAIR, 2 * GPC, HW], BF16)
            res = stage.tile([128, NJ, GPC], BF16)     # column c = ORDER[c]
            qk_sb = stage.tile([C8, GPC, 2, HW], F8)
            pam_sb = stage.tile([128, GPC, 2], BF16)
            kout = stage.tile([NJ, GPC, 128], BF16)
            pout = stage.tile([2 * GPC, 128], BF16)

            # ---------- QAConv helpers ----------
            a_pend = []
            a_second = set()

            def qa_mms(j, ccs):
                """Issue matmuls for j over the given cc list."""
                jc, jl = divmod(j, JPC)
                if PATHS[j] == "D":
                    # slot-major: finish each psum slot's accumulation
                    # before starting the next (interleaved start groups
                    # within one psum bank corrupt each other on HW)
                    tiles = qa_tiles[j]
                    for half in range(2):
                        ps = tiles[half]
                        for gs in range(2):
                            g = half * 2 + gs
                            for cc in ccs:
                                lhsT = xt[cc][jc][:, :,
                                                  jl * 128:(jl + 1) * 128]
                                nc.tensor.matmul(
                                    ps[:, gs, :], lhsT,
                                    xt[cc][0][:, :, g * HW:(g + 1) * HW],
                                    start=(cc == 0), stop=(cc == 1),
                                    perf_mode=DR)
                    return
                # A path: j is the first or second of a pair; moving axis
                # covered in 256-col chunks across 3 full-bank tiles
                first = j not in a_second
                pj = j if first else j - 1
                tiles = qa_tiles[pj]
                half = 0 if first else 1   # which j of the pair
                for ch in range(3):        # this j's three 256-col chunks
                    gch = half * 3 + ch
                    ps = tiles[gch // 2]
                    for cc in ccs:
                        lhsT = xt[cc][jc][:, :, jl * 128:(jl + 1) * 128]
                        nc.tensor.matmul(
                            ps[:, gch % 2, :], lhsT,
                            xt[cc][0][:, :, ch * 256:(ch + 1) * 256],
                            start=(cc == 0), stop=(cc == 1),
                            perf_mode=DR)

            def qa_reduce(j):
                path = PATHS[j]
                col = COL_OF_J[j]
                if path == "D":
                    t0, t1 = qa_tiles[j]
                    nc.vector.tensor_reduce(
                        res[:, col, 0:2, None], t0[:], op=MAX, axis=AX)
                    nc.vector.tensor_reduce(
                        res[:, col, 2:4, None], t1[:], op=MAX, axis=AX)
                    return
                if j in a_second:
                    # second j of the pair: copy the three bank tiles
                    pj = j - 1
                    tiles = qa_tiles[pj]
                    slot = len(a_pend) + sum(len(b) for b in a_batches)
                    flat = sb192[:, slot, :, :].rearrange("p g t -> p (g t)")
                    for i in range(3):
                        nc.scalar.copy(
                            flat[:, i * 512:(i + 1) * 512],
                            tiles[i][:].rearrange("p a b -> p (a b)"))
                    a_pend.append((slot, col - 1))
                    if slot + 1 in A_FLUSH:
                        flush_a()

            a_batches = []

            def flush_a():
                if not a_pend:
                    return
                batch = list(a_pend)
                a_pend.clear()
                a_batches.append(batch)
                s0 = batch[0][0]
                c0 = batch[0][1]
                k = len(batch)
                cview = sb192[:, s0:s0 + k, :, :]
                _tree_max(nc, cview, 192)
                nc.vector.tensor_reduce(
                    res[:, c0:c0 + 2 * k, :, None].rearrange(
                        "p (a b) g w -> p a (b g) w", a=k),
                    cview[:, :, :, 0:3], op=MAX, axis=AX)

            # psum tiles per j (allocated lazily, ring via tag)
            qa_tiles = {}

            def alloc_qa(j):
                if PATHS[j] == "D":
                    qa_tiles[j] = (
                        qad_psum.tile([128, 2, HW], F32, tag="qad",
                                      name=f"qa_{j}_0"),
                        qad_psum.tile([128, 2, HW], F32, tag="qad",
                                      name=f"qa_{j}_1"),
                    )
                elif j not in a_second:
                    a_second.add(j + 1)
                    qa_tiles[j] = tuple(
                        qaa_psum.tile([128, 2, 256], F32, tag="qaa",
                                      name=f"qa_{j}_{i}")
                        for i in range(3))

            # ---------- PAM projections ----------
            # bias folded in as two extra contraction channels (ones/zeros)
            def pam_proj():
                if HAS_BIAS[0]:
                    cst = wpool.tile([1, 2, HW], F8)
                    nc.vector.memset(cst[:, 0, :], 1.0)
                    nc.vector.memset(cst[:, 1, :], 0.0)
                for b in range(GPC):
                    pp = pam_psum.tile([C8, 2, HW], F32, tag="pam",
                                       name=f"proj_{b}")
                    rhs0 = xt[0][0][:, :, b * HW:(b + 1) * HW]
                    rhs1 = xt[1][0][:, :, b * HW:(b + 1) * HW]
                    for qk, w_sb, bw in ((0, wq_sb, bqw_sb), (1, wk_sb, bkw_sb)):
                        nc.tensor.matmul(pp[:, qk, :], w_sb[0][:], rhs0,
                                         start=True, stop=False, perf_mode=DR)
                        nc.tensor.matmul(pp[:, qk, :], w_sb[1][:], rhs1,
                                         start=False,
                                         stop=not HAS_BIAS[0], perf_mode=DR)
                        if HAS_BIAS[0]:
                            nc.tensor.matmul(pp[:, qk, :], bw[:], cst[:],
                                             start=False, stop=True,
                                             perf_mode=DR)
                    if b < 4:
                        nc.vector.tensor_scalar_mul(
                            qk_sb[:, b, :, :], pp[:], 1.0 / WSCALE)
                    else:
                        nc.scalar.activation(qk_sb[:, b, :, :], pp[:], COPY,
                                             bias=0.0, scale=1.0 / WSCALE)

            def pam_out():
                tpp = pam_psum.tile([2 * GPC, 128], BF16, tag="pam",
                                    name="tpp")
                nc.tensor.transpose(
                    tpp[:], pam_sb[:].rearrange("p b h -> p (b h)"),
                    ident_bf[:])
                nc.scalar.copy(pout[:], tpp[:])
                nc.sync.dma_start(pmax_bf[:], pout[:])

            def pam_energy():
                for b in range(GPC):
                    for mch, (m0, msz) in enumerate(((0, 128), (128, 64))):
                        e = pam_psum.tile([128, HW], F32, tag="pam",
                                          name=f"e_{b}_{mch}")
                        nc.tensor.matmul(
                            e[:msz, :], qk_sb[:, b, 0, m0:m0 + msz],
                            qk_sb[:, b, 1, :], start=True, stop=True)
                        nc.vector.tensor_reduce(
                            pam_sb[:msz, b, mch:mch + 1], e[:msz, :],
                            op=MAX, axis=AX)

            # ---------- schedule ----------
            # start QA j0/j1 cc0 as soon as the first x tile lands, then
            # PAM projections (need weights + both cc of ch0)
            alloc_qa(0)
            alloc_qa(1)
            qa_mms(0, [0, 1])
            pam_proj()
            qa_mms(1, [0, 1])
            qa_reduce(0)
            qa_reduce(1)
            for j in range(2, NJ):
                alloc_qa(j)
                qa_mms(j, [0, 1])
                qa_reduce(j)
                if j == 12:
                    pam_energy()
                if j == 20:
                    pam_out()
            flush_a()

            # ---------- output: transpose + copy + DMA ----------
            tpk = pam_psum.tile([NJ, GPC, 128], BF16, tag="pam", name="tpk")
            for g in range(GPC):
                nc.tensor.transpose(tpk[:, g, :], res[:, :, g], ident_bf[:])
            nc.vector.tensor_copy(kout[:], tpk[:])
            nc.sync.dma_start(kmax_bf[:], kout[:])

    nc.compile()
    return nc


def kernel(x, Wq, bq, Wk, bk):
    global _COMPILED
    has_bias = bool(np.any(np.asarray(bq)) or np.any(np.asarray(bk)))
    if _COMPILED is None or _COMPILED[1] != has_bias:
        HAS_BIAS[0] = has_bias
        _COMPILED = (_build(), has_bias)
    nc = _COMPILED[0]

    x = np.ascontiguousarray(x, dtype=np.float32)
    xT = np.ascontiguousarray(
        x.reshape(B, C, HW).transpose(1, 0, 2).reshape(C, FLAT))
    x8 = xT.astype(NP_F8)
    wq8 = np.ascontiguousarray(
        (np.asarray(Wq, np.float32).T * WSCALE)).astype(NP_F8)
    wk8 = np.ascontiguousarray(
        (np.asarray(Wk, np.float32).T * WSCALE)).astype(NP_F8)
    bqa = np.zeros((2, C8), np.float32)
    bqa[0] = np.asarray(bq, np.float32) * WSCALE
    bka = np.zeros((2, C8), np.float32)
    bka[0] = np.asarray(bk, np.float32) * WSCALE
    bq8 = bqa.astype(NP_F8)
    bk8 = bka.astype(NP_F8)

    in_maps = [
        {
            "x8": np.ascontiguousarray(np.roll(x8, -i * GROLL, axis=1)),
            "wq8": wq8,
            "wk8": wk8,
            "bqw": bq8,
            "bkw": bk8,
        }
        for i in range(N_CORES)
    ]

    res = run_bass_kernel_spmd(nc, in_maps, core_ids=list(range(N_CORES)))

    order = np.asarray(ORDER)
    kernel_max = np.empty((B, FLAT), np.float32)
    pos_max = np.empty((B, HW), np.float32)
    for i, r in enumerate(res.results):
        km = np.asarray(r["kmax_bf"]).astype(np.float32).reshape(NJ, GPC, 128)
        km_j = np.empty_like(km)
        km_j[order] = km                        # undo path permutation
        km_flat = km_j.transpose(1, 0, 2).reshape(GPC, FLAT)
        kernel_max[i * GPC:(i + 1) * GPC] = np.roll(
            km_flat, i * GROLL, axis=1)
        pm = np.asarray(r["pmax_bf"]).astype(np.float32).reshape(2 * GPC, 128)
        for b in range(GPC):
            pos_max[i * GPC + b, 0:128] = pm[2 * b]
            pos_max[i * GPC + b, 128:HW] = pm[2 * b + 1, 0:64]

    return np.concatenate(
        [kernel_max.reshape(B, B, HW), pos_max[None]], axis=0
    ).astype(np.float32)
